# revision 29
# baseline (speedup 1.0000x reference)
"""Trainium2 Bass kernel for nn_BidPrefix (segment_reduce).

Problem: inputs [B=500000, 302] f32 rows = [rates[0:300], market_price, bid].
  cp1[k] = prod(rates[:k])  (exclusive prefix products, cp1[0] = 1)
  survival  = cp1[bid]
  rate_last = cp1[mp] - cp1[mp+1]

Strategy (pure data parallel over 8 NeuronCores, batch sharded):
  R=8 row-blocks per partition: each 128-partition tile covers 1024 rows,
  loaded as one DMA of [128, 8*302] (8 contiguous 1208B runs/partition).

  Per tile:
   - DVE computes ONE affine prefix scan over all 8 blocks at once:
       state = state*d0[t] + d1[t]
     where d0 is the tile (bid columns zeroed by ACT so the state dies at
     each block boundary) and d1 is a constant one-hot (1.0 at each bid
     column) that restarts the state at 1.0 for the next block. The scan
     output is exactly the 8 blocks' exclusive cumprods, concatenated,
     with cp1-block k at buf cols [302k, 302k+300].
   - Small DVE ops build per-row gather indices (mp, mp+1, bid) + 302k
     block offsets, cast to int16.
   - GPSIMD ap_gather (sole Pool op; ap_gather ucode library loaded once)
     pulls all 3 gathers for 16 rows x 8 blocks per partition group in a
     single instruction: out[p, s*16+l] = buf[p, idx[16g+l, s]].
   - Row p only owns slots with l == p%16: one DVE mask-multiply against
     a constant 0/1 mask + one segmented tensor_reduce extracts the
     [128, 8, 3] results per tile into persistent accumulators.
  Tail: rate_last = A_mp - A_mp1 in one wide subtract; outputs stored as
  [128, T*R] per core, un-interleaved on the host (a.T.reshape).

DVE does ~466ns/128rows (scan-dominated), matching the DMA roofline of
466ns/128rows; ACT/GPSIMD stay under it. All constants (reset one-hot,
mask, offsets) are tiny ExternalInputs so GPSIMD never reloads ucode.
"""

import numpy as np

SEQ = 300
W = SEQ + 2  # 302 input columns per row
B = 500000
N_CORES = 8
R = 8  # row-blocks per partition
TILE_ROWS = 128 * R  # 1024
T = 62  # tiles per core
ROWS_PER_CORE = T * TILE_ROWS  # 63488; 8*63488 = 507904 >= B
WIDE = R * W  # 2416
SCAN_W = (R - 1) * W + SEQ  # 2414: cols 0..2413 feed the scan
NE = SCAN_W + 1  # 2415 gather num_elems (buf cols 0..2414)
NSLOT = 3 * R  # 24 index slots per partition
NIDX = 16 * NSLOT  # 384 gathered values per partition
OUT_COLS = T * R  # 496

_CACHE = {}


def _split_multi_waits(nc, max_waits=1):
    """Walrus in this container rejects instructions with >1 sync-wait.

    Hoist extra waits onto single-wait NOPs inserted right before the
    offending instruction on the same engine (same-queue program order
    preserves semantics).
    """
    import concourse.mybir as mybir

    ctr = 0
    for fn in nc.m.functions:
        for bb in fn.blocks:
            il = bb.instructions
            i = 0
            while i < len(il):
                ins = il[i]
                si = ins.sync_info
                if si is not None and si.on_wait and len(si.on_wait) > max_waits:
                    waits = list(si.on_wait)
                    pos = i
                    for w in waits[max_waits:]:
                        ctr += 1
                        nop = mybir.InstNoOp(
                            name=f"I-splitwait-{ctr}",
                            engine=ins.engine,
                            sync_info=mybir.SyncInfo(on_wait=[w], on_update=[]),
                        )
                        il.insert(pos, nop)
                        pos += 1
                        i += 1
                    si.on_wait = waits[:max_waits]
                i += 1


def make_consts():
    """Host-built constant tensors shipped to every core."""
    wm = np.zeros((128, NIDX), np.float32)
    for p in range(128):
        wm[p, np.arange(NSLOT) * 16 + p % 16] = 1.0
    rst = np.zeros((128, SCAN_W), np.float32)
    for k in range(R - 1):
        rst[:, k * W + SEQ + 1] = 1.0
    offa = np.zeros((128, R, 2), np.float32)
    offb = np.zeros((128, R), np.float32)
    for k in range(R):
        offa[:, k, :] = k * W
        offb[:, k] = k * W + 1
    return {"wm": wm, "rst": rst, "offa": offa, "offb": offb}


def _build_nc(
    in_bufs=4, g_bufs=3, repeat=1, scan=1, gather=1, extract_tt=1, dma_only=0
):
    import concourse.bass as bass
    import concourse.tile as tile
    from concourse import mybir, library_config

    F32 = mybir.dt.float32
    I16 = mybir.dt.int16

    nc = bass.Bass("TRN2")
    x = nc.dram_tensor("inputs", [ROWS_PER_CORE, W], F32, kind="ExternalInput")
    c_wm = nc.dram_tensor("wm", [128, NIDX], F32, kind="ExternalInput")
    c_rst = nc.dram_tensor("rst", [128, SCAN_W], F32, kind="ExternalInput")
    c_offa = nc.dram_tensor("offa", [128, R, 2], F32, kind="ExternalInput")
    c_offb = nc.dram_tensor("offb", [128, R], F32, kind="ExternalInput")
    out_s = nc.dram_tensor("surv", [128, OUT_COLS], F32, kind="ExternalOutput")
    out_r = nc.dram_tensor("ratelast", [128, OUT_COLS], F32, kind="ExternalOutput")

    x_t = x.rearrange("(t r p) c -> t p r c", p=128, r=R)

    with tile.TileContext(nc) as tc:
        with (
            tc.tile_pool(name="inp", bufs=in_bufs) as inp_pool,
            tc.tile_pool(name="gat", bufs=g_bufs) as gat_pool,
            tc.tile_pool(name="idx", bufs=3) as idx_pool,
            tc.tile_pool(name="persist", bufs=1) as persist,
        ):
            WM = persist.tile([128, NIDX], F32, tag="wm")
            RST = persist.tile([128, SCAN_W], F32, tag="rst")
            OFFA = persist.tile([128, R, 2], F32, tag="offa")
            OFFB = persist.tile([128, R], F32, tag="offb")
            nc.sync.dma_start(out=WM[:, :], in_=c_wm[:, :])
            nc.sync.dma_start(out=RST[:, :], in_=c_rst[:, :])
            nc.sync.dma_start(out=OFFA[:, :, :], in_=c_offa[:, :, :])
            nc.sync.dma_start(out=OFFB[:, :], in_=c_offb[:, :])

            nc.gpsimd.load_library(library_config.ap_gather)

            ACC = persist.tile([128, T * R, 3], F32, tag="acc")
            RL = persist.tile([128, OUT_COLS], F32, tag="rl")

            # rotating cp1 buffers; col 0 preset to 1.0 once
            n_cp = 3
            cp_bufs = []
            for j in range(n_cp):
                t = persist.tile([128, NE], F32, tag=f"cp1_{j}")
                if scan:
                    nc.vector.memset(t[:, 0:1], 1.0)
                else:
                    nc.vector.memset(t[:, :], 1.0)
                cp_bufs.append(t)

            if dma_only:
                nc.vector.memset(ACC[:, :, :], 0.0)

            for i in [i for _ in range(repeat) for i in range(T)]:
                xt = inp_pool.tile([128, WIDE], F32, tag="xt")
                xt3 = xt[:, :].rearrange("p (r c) -> p r c", r=R)
                nc.sync.dma_start(out=xt3, in_=x_t[i, :, :, :])
                if dma_only:
                    continue

                # per-row gather indices (mp, mp+1, bid) + block offsets
                idxf = idx_pool.tile([128, R, 3], F32, tag="idxf")
                nc.vector.scalar_tensor_tensor(
                    out=idxf[:, :, 0:3:2],
                    in0=xt3[:, :, SEQ : SEQ + 2],
                    scalar=0.0,
                    in1=OFFA[:, :, :],
                    op0=mybir.AluOpType.add,
                    op1=mybir.AluOpType.add,
                )
                nc.vector.scalar_tensor_tensor(
                    out=idxf[:, :, 1:2],
                    in0=xt3[:, :, SEQ : SEQ + 1],
                    scalar=0.0,
                    in1=OFFB[:, :, None],
                    op0=mybir.AluOpType.add,
                    op1=mybir.AluOpType.add,
                )
                idx16 = idx_pool.tile([128, NSLOT], I16, tag="idx16")
                nc.scalar.copy(
                    idx16[:, :], idxf[:, :, :].rearrange("p a b -> p (a b)")
                )

                # kill the scan state at block boundaries (bid columns)
                nc.scalar.mul(
                    xt[:, SEQ + 1 : SCAN_W : W], xt[:, SEQ + 1 : SCAN_W : W], 0.0
                )

                buf = cp_bufs[i % n_cp]
                if scan:
                    nc.vector.tensor_tensor_scan(
                        out=buf[:, 1:NE],
                        data0=xt[:, 0:SCAN_W],
                        data1=RST[:, :],
                        initial=1.0,
                        op0=mybir.AluOpType.mult,
                        op1=mybir.AluOpType.add,
                    )

                if gather:
                    G = gat_pool.tile([128, NIDX], F32, tag="g")
                    nc.gpsimd.ap_gather(
                        G[:, :], buf[:, :], idx16[:, :],
                        channels=128, num_elems=NE, d=1, num_idxs=NIDX,
                    )
                    g_in = G[:, :]
                else:
                    g_in = WM[:, :]

                if extract_tt:
                    GM = gat_pool.tile([128, NIDX], F32, tag="gm")
                    nc.vector.tensor_tensor(
                        GM[:, :], g_in, WM[:, :], mybir.AluOpType.mult
                    )
                    red_in = GM[:, :]
                else:
                    red_in = g_in
                nc.vector.tensor_reduce(
                    out=ACC[:, i * R : (i + 1) * R, :],
                    in_=red_in.rearrange("p (s l) -> p s l", l=16),
                    op=mybir.AluOpType.add,
                    axis=mybir.AxisListType.X,
                )

            nc.vector.tensor_tensor(
                RL[:, :], ACC[:, :, 0], ACC[:, :, 1], mybir.AluOpType.subtract
            )
            nc.sync.dma_start(out=out_s[:, :], in_=ACC[:, :, 2])
            nc.sync.dma_start(out=out_r[:, :], in_=RL[:, :])

    _split_multi_waits(nc)
    # Raw Bass skips Bacc's codegen pass that fills in .instr bytes for
    # extended-ISA instructions (ap_gather, library load); without it the
    # NEFF compiler fails with "ISA wrong length".
    from concourse.library_overlay import lower_extended_insts

    lower_extended_insts(nc)
    return nc


def make_consts_log():
    iota = np.tile(np.arange(SEQ, dtype=np.float32), (128, 1))
    return {"iotaf": iota}


def _build_nc_log(repeat=1, pool=1, in_bufs=6, l_bufs=4, tiles=None):
    """Log-space architecture: no cumprod scan, no gathers.

    Per 128-row tile: ACT computes L = ln(rates); three one-pass STT
    masked accumulations produce s_bid = sum(L[j<bid]), s_mp =
    sum(L[j<mp]) and r_mp = rates[mp]; tail computes exp() on ACT and
    rate_last = exp(s_mp)*(1-r_mp). `pool` (int or digit-string cycle)
    sets how many of the 3 STTs run on GPSIMD instead of DVE per tile.
    The r_mp STT only needs the raw tile so it runs in parallel with the
    Ln; the two masked-L sums depend on the Ln output.
    """
    import concourse.bass as bass
    import concourse.tile as tile
    from concourse import mybir

    F32 = mybir.dt.float32
    TT = tiles or T_LOG
    pattern = [int(c) for c in str(pool)]

    nc = bass.Bass("TRN2")
    x = nc.dram_tensor("inputs", [TT * 128, W], F32, kind="ExternalInput")
    c_iota = nc.dram_tensor("iotaf", [128, SEQ], F32, kind="ExternalInput")
    out_s = nc.dram_tensor("surv", [128, TT], F32, kind="ExternalOutput")
    out_r = nc.dram_tensor("ratelast", [128, TT], F32, kind="ExternalOutput")

    x_t = x.rearrange("(t p) c -> t p c", p=128)

    with tile.TileContext(nc) as tc:
        with (
            tc.tile_pool(name="inp", bufs=in_bufs) as inp_pool,
            tc.tile_pool(name="lp", bufs=l_bufs) as l_pool,
            tc.tile_pool(name="persist", bufs=1) as persist,
        ):
            IOTA = persist.tile([128, SEQ], F32, tag="iota")
            nc.sync.dma_start(out=IOTA[:, :], in_=c_iota[:, :])

            SB = persist.tile([128, TT], F32, tag="sb")
            SM = persist.tile([128, TT], F32, tag="sm")
            RM = persist.tile([128, TT], F32, tag="rm")

            for i in [i for _ in range(repeat) for i in range(TT)]:
                xt = inp_pool.tile([128, W], F32, tag="xt")
                nc.sync.dma_start(out=xt[:, :], in_=x_t[i, :, :])
                rates = xt[:, 0:SEQ]
                mp = xt[:, SEQ : SEQ + 1]
                bid = xt[:, SEQ + 1 : SEQ + 2]

                L = l_pool.tile([128, SEQ], F32, tag="L")
                nc.scalar.activation(
                    L[:, :], rates, mybir.ActivationFunctionType.Ln
                )

                n_pool = pattern[i % len(pattern)]
                tr1 = l_pool.tile([128, SEQ], F32, tag="tr1")
                tr2 = l_pool.tile([128, SEQ], F32, tag="tr2")
                tr3 = l_pool.tile([128, SEQ], F32, tag="tr3")
                # (engine, in0, scalar, in1, op0, acc, out-trash)
                jobs = [
                    # r_mp: raw-tile dependent only -> Pool first choice
                    (IOTA[:, :], mp, rates, mybir.AluOpType.is_equal, RM, tr1),
                    # s_bid
                    (IOTA[:, :], bid, L[:, :], mybir.AluOpType.is_lt, SB, tr2),
                    # s_mp
                    (IOTA[:, :], mp, L[:, :], mybir.AluOpType.is_lt, SM, tr3),
                ]
                for j, (in0, sc, in1, op0, acc, tr) in enumerate(jobs):
                    eng = nc.gpsimd if j < n_pool else nc.vector
                    eng.scalar_tensor_tensor(
                        out=tr[:, :],
                        in0=in0,
                        scalar=sc,
                        in1=in1,
                        op0=op0,
                        op1=mybir.AluOpType.mult,
                        accum_out=acc[:, i : i + 1],
                    )

            ES = persist.tile([128, TT], F32, tag="es")
            nc.scalar.activation(ES[:, :], SB[:, :], mybir.ActivationFunctionType.Exp)
            EM = persist.tile([128, TT], F32, tag="em")
            nc.scalar.activation(EM[:, :], SM[:, :], mybir.ActivationFunctionType.Exp)
            OM = persist.tile([128, TT], F32, tag="om")
            nc.vector.tensor_scalar(
                out=OM[:, :], in0=RM[:, :], scalar1=-1.0, scalar2=1.0,
                op0=mybir.AluOpType.mult, op1=mybir.AluOpType.add,
            )
            RL = persist.tile([128, TT], F32, tag="rlt")
            nc.vector.tensor_tensor(RL[:, :], EM[:, :], OM[:, :], mybir.AluOpType.mult)
            nc.sync.dma_start(out=out_s[:, :], in_=ES[:, :])
            nc.sync.dma_start(out=out_r[:, :], in_=RL[:, :])

    _split_multi_waits(nc)
    from concourse.library_overlay import lower_extended_insts

    lower_extended_insts(nc)
    return nc


T_LOG = 489  # tiles per core for the log architecture; 489*128 = 62592


def make_consts_f():
    iota = np.tile(np.arange(SEQ, dtype=np.float32), (128, 1))
    wmc = np.zeros((128, 128), np.float32)
    for p in range(128):
        wmc[p, np.arange(R) * 16 + p % 16] = 1.0
    offc = np.zeros((128, R), np.float32)
    for k in range(R):
        offc[:, k] = k * W
    return {"iotaf": iota, "wmc": wmc, "offc": offc}


def _build_nc_f(repeat=1, in_bufs=5, l_bufs=4, g_bufs=4):
    """Arch F: log-space masked sums on DVE + raw-tile r_mp gather on Pool.

    Per tile (R=8 row-blocks, 1024 rows):
      ACT:  L[k] = ln(rates block k)  (one wide strided op) + idx cast
      DVE:  16 STT masked sums: s_bid[k] = sum(L_k[j<bid_k]),
            s_mp[k] = sum(L_k[j<mp_k]); + extraction of the gather
      Pool: one ap_gather pulls r_mp[k] = xt[302k + mp_k] for all 16
            rows x 8 blocks of each partition group (128 idx)
      tail (per tile, tiny): surv = exp(s_bid); rate_last =
            exp(s_mp)*(1 - r_mp)
    """
    import concourse.bass as bass
    import concourse.tile as tile
    from concourse import mybir, library_config

    F32 = mybir.dt.float32
    I16 = mybir.dt.int16
    NID = 16 * R  # 128 gather slots per partition-group

    nc = bass.Bass("TRN2")
    x = nc.dram_tensor("inputs", [ROWS_PER_CORE, W], F32, kind="ExternalInput")
    c_iota = nc.dram_tensor("iotaf", [128, SEQ], F32, kind="ExternalInput")
    c_wmc = nc.dram_tensor("wmc", [128, 128], F32, kind="ExternalInput")
    c_offc = nc.dram_tensor("offc", [128, R], F32, kind="ExternalInput")
    out_s = nc.dram_tensor("surv", [128, OUT_COLS], F32, kind="ExternalOutput")
    out_r = nc.dram_tensor("ratelast", [128, OUT_COLS], F32, kind="ExternalOutput")

    x_t = x.rearrange("(t r p) c -> t p r c", p=128, r=R)

    with tile.TileContext(nc) as tc:
        with (
            tc.tile_pool(name="inp", bufs=in_bufs) as inp_pool,
            tc.tile_pool(name="lp", bufs=l_bufs) as l_pool,
            tc.tile_pool(name="gat", bufs=g_bufs) as gat_pool,
            tc.tile_pool(name="persist", bufs=1) as persist,
        ):
            IOTA = persist.tile([128, SEQ], F32, tag="iota")
            WMC = persist.tile([128, 128], F32, tag="wmc")
            OFFC = persist.tile([128, R], F32, tag="offc")
            nc.sync.dma_start(out=IOTA[:, :], in_=c_iota[:, :])
            nc.sync.dma_start(out=WMC[:, :], in_=c_wmc[:, :])
            nc.sync.dma_start(out=OFFC[:, :], in_=c_offc[:, :])

            nc.gpsimd.load_library(library_config.ap_gather)

            SB = persist.tile([128, T * R], F32, tag="sb")
            SM = persist.tile([128, T * R], F32, tag="sm")
            OUTS = persist.tile([128, OUT_COLS], F32, tag="outs")
            OUTR = persist.tile([128, OUT_COLS], F32, tag="outr")

            for i in [i for _ in range(repeat) for i in range(T)]:
                xt = inp_pool.tile([128, WIDE], F32, tag="xt")
                xt3 = xt[:, :].rearrange("p (r c) -> p r c", r=R)
                nc.sync.dma_start(out=xt3, in_=x_t[i, :, :, :])

                # L = ln(rates), all 8 blocks in one strided ACT op
                L = l_pool.tile([128, R, SEQ], F32, tag="L")
                nc.scalar.activation(
                    L[:, :, :], xt3[:, :, 0:SEQ], mybir.ActivationFunctionType.Ln
                )

                # r_mp gather indices: 302k + mp_k, cast on ACT
                idxf = gat_pool.tile([128, R], F32, tag="idxf")
                nc.vector.scalar_tensor_tensor(
                    out=idxf[:, :], in0=xt[:, SEQ : WIDE : W], scalar=0.0,
                    in1=OFFC[:, :], op0=mybir.AluOpType.add,
                    op1=mybir.AluOpType.add,
                )
                idx16 = gat_pool.tile([128, R], I16, tag="idx16")
                nc.scalar.copy(idx16[:, :], idxf[:, :])

                G = gat_pool.tile([128, NID], F32, tag="g")
                nc.gpsimd.ap_gather(
                    G[:, :], xt[:, :], idx16[:, :],
                    channels=128, num_elems=WIDE, d=1, num_idxs=NID,
                )

                # 16 masked-sum STTs (the DVE workhorse)
                tra = l_pool.tile([128, SEQ], F32, tag="tra")
                trb = l_pool.tile([128, SEQ], F32, tag="trb")
                for k in range(R):
                    nc.vector.scalar_tensor_tensor(
                        out=(tra if k % 2 else trb)[:, :],
                        in0=IOTA[:, :],
                        scalar=xt[:, k * W + SEQ + 1 : k * W + SEQ + 2],
                        in1=L[:, k, :],
                        op0=mybir.AluOpType.is_lt,
                        op1=mybir.AluOpType.mult,
                        accum_out=SB[:, i * R + k : i * R + k + 1],
                    )
                for k in range(R):
                    nc.vector.scalar_tensor_tensor(
                        out=(tra if k % 2 else trb)[:, :],
                        in0=IOTA[:, :],
                        scalar=xt[:, k * W + SEQ : k * W + SEQ + 1],
                        in1=L[:, k, :],
                        op0=mybir.AluOpType.is_lt,
                        op1=mybir.AluOpType.mult,
                        accum_out=SM[:, i * R + k : i * R + k + 1],
                    )

                # extract r_mp from the wrapped gather
                GM = gat_pool.tile([128, NID], F32, tag="gm")
                nc.vector.tensor_tensor(
                    GM[:, :], G[:, :], WMC[:, :], mybir.AluOpType.mult
                )
                RMt = gat_pool.tile([128, R], F32, tag="rmt")
                nc.vector.tensor_reduce(
                    out=RMt[:, :],
                    in_=GM[:, :].rearrange("p (s l) -> p s l", l=16),
                    op=mybir.AluOpType.add,
                    axis=mybir.AxisListType.X,
                )

                # tail: surv = exp(s_bid); rate_last = exp(s_mp) * (1 - r_mp)
                sl = slice(i * R, (i + 1) * R)
                nc.scalar.activation(
                    OUTS[:, sl], SB[:, sl], mybir.ActivationFunctionType.Exp
                )
                EM = gat_pool.tile([128, R], F32, tag="em")
                nc.scalar.activation(
                    EM[:, :], SM[:, sl], mybir.ActivationFunctionType.Exp
                )
                OM = gat_pool.tile([128, R], F32, tag="om")
                nc.scalar.activation(
                    OM[:, :], RMt[:, :], mybir.ActivationFunctionType.Copy,
                    bias=1.0, scale=-1.0,
                )
                nc.vector.tensor_tensor(
                    OUTR[:, sl], EM[:, :], OM[:, :], mybir.AluOpType.mult
                )

            nc.sync.dma_start(out=out_s[:, :], in_=OUTS[:, :])
            nc.sync.dma_start(out=out_r[:, :], in_=OUTR[:, :])

    _split_multi_waits(nc)
    from concourse.library_overlay import lower_extended_insts

    lower_extended_insts(nc)
    return nc


def _build_nc_micro(repeat=1, op=0, width=300, n=200, nbuf=4, d=1):
    """Microbenchmark: `repeat*n` back-to-back instructions of one kind.

    op: 0=STT(is_lt,mult,accum) 1=scan 2=TT(mult) 3=reduce(seg16)
        4=ACT Ln  5=ap_gather(num_idxs=width)  6=tensor_copy
    """
    import concourse.bass as bass
    import concourse.tile as tile
    from concourse import mybir, library_config

    F32 = mybir.dt.float32
    I16 = mybir.dt.int16

    nc = bass.Bass("TRN2")
    x = nc.dram_tensor("inputs", [128, W], F32, kind="ExternalInput")
    out_s = nc.dram_tensor("surv", [128, 1], F32, kind="ExternalOutput")
    out_r = nc.dram_tensor("ratelast", [128, 1], F32, kind="ExternalOutput")

    with tile.TileContext(nc) as tc:
        with tc.tile_pool(name="p", bufs=1) as pool:
            A = pool.tile([128, max(width, 2415)], F32, tag="a")
            B = pool.tile([128, max(width, 2415)], F32, tag="b")
            S = pool.tile([128, 1], F32, tag="s")
            nc.sync.dma_start(out=A[:, 0:W], in_=x[:, :])
            nc.vector.memset(A[:, W:], 0.5)
            nc.vector.memset(B[:, :], 0.5)
            nc.vector.memset(S[:, :], 3.0)
            IDX = pool.tile([128, max(width // 16, 4)], I16, tag="idx")
            nc.vector.memset(IDX[:, :], 7)
            if op == 5:
                nc.gpsimd.load_library(library_config.ap_gather)
            trash = []
            acc = []
            for j in range(nbuf):
                tr = pool.tile([128, width * d], F32, tag=f"tr{j}")
                trash.append(tr)
                ac = pool.tile([128, 1], F32, tag=f"ac{j}")
                nc.vector.memset(ac[:, :], 0.0)
                nc.vector.memset(tr[:, :], 0.0)
                acc.append(ac)
            for i in range(repeat * n):
                tr = trash[i % nbuf]
                ac = acc[i % nbuf]
                if op == 0:
                    nc.vector.scalar_tensor_tensor(
                        out=tr[:, :], in0=A[:, 0:width], scalar=S[:, :],
                        in1=B[:, 0:width], op0=mybir.AluOpType.is_lt,
                        op1=mybir.AluOpType.mult, accum_out=ac[:, :],
                    )
                elif op == 1:
                    nc.vector.tensor_tensor_scan(
                        out=tr[:, :], data0=A[:, 0:width], data1=B[:, 0:width],
                        initial=1.0, op0=mybir.AluOpType.mult,
                        op1=mybir.AluOpType.add,
                    )
                elif op == 2:
                    nc.vector.tensor_tensor(
                        tr[:, :], A[:, 0:width], B[:, 0:width], mybir.AluOpType.mult
                    )
                elif op == 3:
                    nc.vector.tensor_reduce(
                        out=tr[:, 0 : width // 16],
                        in_=A[:, 0:width].rearrange("p (s l) -> p s l", l=16),
                        op=mybir.AluOpType.add, axis=mybir.AxisListType.X,
                    )
                elif op == 4:
                    nc.scalar.activation(
                        tr[:, :], A[:, 0:width], mybir.ActivationFunctionType.Ln
                    )
                elif op == 5:
                    nc.gpsimd.ap_gather(
                        tr[:, :],
                        B[:, 0 : (2415 // d) * d].rearrange(
                            "p (e dd) -> p e dd", dd=d
                        ),
                        IDX[:, 0 : width // 16],
                        channels=128, num_elems=2415 // d, d=d, num_idxs=width,
                    )
                elif op == 6:
                    nc.vector.tensor_copy(tr[:, :], A[:, 0:width])
            nc.sync.dma_start(out=out_s[:, :], in_=acc[0][:, :])
            nc.sync.dma_start(out=out_r[:, :], in_=trash[0][:, 0:1])

    _split_multi_waits(nc)
    from concourse.library_overlay import lower_extended_insts

    lower_extended_insts(nc)
    return nc


# --- public kernel() configuration ---
# "log":  log-space masked-sum architecture (_build_nc_log)
# "scan": R=8 scan + ap_gather architecture (_build_nc)
KCONFIG = "f"


def _active_rows_per_core():
    return T_LOG * 128 if KCONFIG == "log" else ROWS_PER_CORE


def _build_active(repeat=1):
    if KCONFIG == "log":
        return _build_nc_log(pool=0, repeat=repeat)
    if KCONFIG == "f":
        return _build_nc_f(repeat=repeat)
    return _build_nc(repeat=repeat)


def _get_nc():
    key = f"nc_{KCONFIG}"
    if key not in _CACHE:
        _CACHE[key] = _build_active()
    return _CACHE[key]


def _shard_inputs(inputs, rows_per_core=None):
    rpc = rows_per_core or _active_rows_per_core()
    total = N_CORES * rpc
    padded = np.empty((total, W), dtype=np.float32)
    padded[: inputs.shape[0]] = inputs
    if total > inputs.shape[0]:
        padded[inputs.shape[0] :, :SEQ] = 1.0
        padded[inputs.shape[0] :, SEQ:] = 0.0
    return [padded[c * rpc : (c + 1) * rpc] for c in range(N_CORES)]


def _active_consts():
    if KCONFIG == "log":
        return make_consts_log()
    if KCONFIG == "f":
        return make_consts_f()
    return make_consts()


def kernel(inputs: np.ndarray):
    from concourse.bass_utils import run_bass_kernel_spmd

    inputs = np.ascontiguousarray(inputs, dtype=np.float32)
    assert inputs.shape == (B, W), inputs.shape

    nc = _get_nc()
    shards = _shard_inputs(inputs)
    consts = _active_consts()
    res = run_bass_kernel_spmd(
        nc,
        [{"inputs": s, **consts} for s in shards],
        core_ids=list(range(N_CORES)),
    )
    surv = np.concatenate(
        [r["surv"].T.reshape(-1, 1) for r in res.results], axis=0
    )[:B]
    rl = np.concatenate(
        [r["ratelast"].T.reshape(-1, 1) for r in res.results], axis=0
    )[:B]
    return surv, rl


# revision 41
# speedup vs baseline: 7.1936x; 7.1936x over previous
"""Trainium2 Bass kernel for nn_BidPrefix (segment_reduce).

Problem: inputs [B=500000, 302] f32 rows = [rates[0:300], market_price, bid].
  cp1[k] = prod(rates[:k])  (exclusive prefix products, cp1[0] = 1)
  survival  = cp1[bid]
  rate_last = cp1[mp] - cp1[mp+1]

Final architecture (KCONFIG="log", _build_nc_log, pure data parallel over
8 NeuronCores, batch sharded):

  Work per 128-row tile, in LOG space (no cumprod scan, no gathers):
    - one DMA loads the [128, 302] tile;
    - ACT computes L = ln(rates) (otherwise-idle engine);
    - three DVE scalar_tensor_tensor one-pass masked accumulations --
      the per-row f32 mp/bid columns are used directly as the STT's
      per-partition scalar operand (no index prep, no casts):
        r_mp  = sum(rates * [j == mp])   (exact rates[mp])
        s_bid = sum(L * [j < bid])
        s_mp  = sum(L * [j < mp])
    - tail (once per core): survival = exp(s_bid);
      rate_last = exp(s_mp) * (1 - r_mp).

  DVE is the bottleneck at 3 passes x 300 elems per row; ACT (ln/exp)
  and DMA stay under its shadow. Measured ~434-530us per core-pass vs
  1.41ms for the staged baseline.

  Rejected alternatives (measured): cumprod scan + gpsimd ap_gather
  extraction (ap_gather costs ~26ns per wrapped index instance and its
  Q7 SBUF traffic slows DVE/DMA by ~75%% of its runtime even when fully
  decoupled); STT on GPSIMD (walrus rejects Pool TensorScalarPtr);
  ACT Relu-ramp masked sums (f32 accumulation drowns the log-signal).

Numerics: ln/exp round-trip gives norm_rel ~3e-6 (gate is 2e-2). The
rate_last form exp(s_mp)*(1-r_mp) avoids subtractive cancellation.
"""

import numpy as np

SEQ = 300
W = SEQ + 2  # 302 input columns per row
B = 500000
N_CORES = 8
R = 8  # row-blocks per partition
TILE_ROWS = 128 * R  # 1024
T = 62  # tiles per core
ROWS_PER_CORE = T * TILE_ROWS  # 63488; 8*63488 = 507904 >= B
WIDE = R * W  # 2416
SCAN_W = (R - 1) * W + SEQ  # 2414: cols 0..2413 feed the scan
NE = SCAN_W + 1  # 2415 gather num_elems (buf cols 0..2414)
NSLOT = 3 * R  # 24 index slots per partition
NIDX = 16 * NSLOT  # 384 gathered values per partition
OUT_COLS = T * R  # 496

_CACHE = {}


def _split_multi_waits(nc, max_waits=1):
    """Walrus in this container rejects instructions with >1 sync-wait.

    Hoist extra waits onto single-wait NOPs inserted right before the
    offending instruction on the same engine (same-queue program order
    preserves semantics).
    """
    import concourse.mybir as mybir

    ctr = 0
    for fn in nc.m.functions:
        for bb in fn.blocks:
            il = bb.instructions
            i = 0
            while i < len(il):
                ins = il[i]
                si = ins.sync_info
                if si is not None and si.on_wait and len(si.on_wait) > max_waits:
                    waits = list(si.on_wait)
                    pos = i
                    for w in waits[max_waits:]:
                        ctr += 1
                        nop = mybir.InstNoOp(
                            name=f"I-splitwait-{ctr}",
                            engine=ins.engine,
                            sync_info=mybir.SyncInfo(on_wait=[w], on_update=[]),
                        )
                        il.insert(pos, nop)
                        pos += 1
                        i += 1
                    si.on_wait = waits[:max_waits]
                i += 1


def make_consts():
    """Host-built constant tensors shipped to every core."""
    wm = np.zeros((128, NIDX), np.float32)
    for p in range(128):
        wm[p, np.arange(NSLOT) * 16 + p % 16] = 1.0
    rst = np.zeros((128, SCAN_W), np.float32)
    for k in range(R - 1):
        rst[:, k * W + SEQ + 1] = 1.0
    offa = np.zeros((128, R, 2), np.float32)
    offb = np.zeros((128, R), np.float32)
    for k in range(R):
        offa[:, k, :] = k * W
        offb[:, k] = k * W + 1
    return {"wm": wm, "rst": rst, "offa": offa, "offb": offb}


def _build_nc(
    in_bufs=4, g_bufs=3, repeat=1, scan=1, gather=1, extract_tt=1, dma_only=0
):
    import concourse.bass as bass
    import concourse.tile as tile
    from concourse import mybir, library_config

    F32 = mybir.dt.float32
    I16 = mybir.dt.int16

    nc = bass.Bass("TRN2")
    x = nc.dram_tensor("inputs", [ROWS_PER_CORE, W], F32, kind="ExternalInput")
    c_wm = nc.dram_tensor("wm", [128, NIDX], F32, kind="ExternalInput")
    c_rst = nc.dram_tensor("rst", [128, SCAN_W], F32, kind="ExternalInput")
    c_offa = nc.dram_tensor("offa", [128, R, 2], F32, kind="ExternalInput")
    c_offb = nc.dram_tensor("offb", [128, R], F32, kind="ExternalInput")
    out_s = nc.dram_tensor("surv", [128, OUT_COLS], F32, kind="ExternalOutput")
    out_r = nc.dram_tensor("ratelast", [128, OUT_COLS], F32, kind="ExternalOutput")

    x_t = x.rearrange("(t r p) c -> t p r c", p=128, r=R)

    with tile.TileContext(nc) as tc:
        with (
            tc.tile_pool(name="inp", bufs=in_bufs) as inp_pool,
            tc.tile_pool(name="gat", bufs=g_bufs) as gat_pool,
            tc.tile_pool(name="idx", bufs=3) as idx_pool,
            tc.tile_pool(name="persist", bufs=1) as persist,
        ):
            WM = persist.tile([128, NIDX], F32, tag="wm")
            RST = persist.tile([128, SCAN_W], F32, tag="rst")
            OFFA = persist.tile([128, R, 2], F32, tag="offa")
            OFFB = persist.tile([128, R], F32, tag="offb")
            nc.sync.dma_start(out=WM[:, :], in_=c_wm[:, :])
            nc.sync.dma_start(out=RST[:, :], in_=c_rst[:, :])
            nc.sync.dma_start(out=OFFA[:, :, :], in_=c_offa[:, :, :])
            nc.sync.dma_start(out=OFFB[:, :], in_=c_offb[:, :])

            nc.gpsimd.load_library(library_config.ap_gather)

            ACC = persist.tile([128, T * R, 3], F32, tag="acc")
            RL = persist.tile([128, OUT_COLS], F32, tag="rl")

            # rotating cp1 buffers; col 0 preset to 1.0 once
            n_cp = 3
            cp_bufs = []
            for j in range(n_cp):
                t = persist.tile([128, NE], F32, tag=f"cp1_{j}")
                if scan:
                    nc.vector.memset(t[:, 0:1], 1.0)
                else:
                    nc.vector.memset(t[:, :], 1.0)
                cp_bufs.append(t)

            if dma_only:
                nc.vector.memset(ACC[:, :, :], 0.0)

            for i in [i for _ in range(repeat) for i in range(T)]:
                xt = inp_pool.tile([128, WIDE], F32, tag="xt")
                xt3 = xt[:, :].rearrange("p (r c) -> p r c", r=R)
                nc.sync.dma_start(out=xt3, in_=x_t[i, :, :, :])
                if dma_only:
                    continue

                # per-row gather indices (mp, mp+1, bid) + block offsets
                idxf = idx_pool.tile([128, R, 3], F32, tag="idxf")
                nc.vector.scalar_tensor_tensor(
                    out=idxf[:, :, 0:3:2],
                    in0=xt3[:, :, SEQ : SEQ + 2],
                    scalar=0.0,
                    in1=OFFA[:, :, :],
                    op0=mybir.AluOpType.add,
                    op1=mybir.AluOpType.add,
                )
                nc.vector.scalar_tensor_tensor(
                    out=idxf[:, :, 1:2],
                    in0=xt3[:, :, SEQ : SEQ + 1],
                    scalar=0.0,
                    in1=OFFB[:, :, None],
                    op0=mybir.AluOpType.add,
                    op1=mybir.AluOpType.add,
                )
                idx16 = idx_pool.tile([128, NSLOT], I16, tag="idx16")
                nc.scalar.copy(
                    idx16[:, :], idxf[:, :, :].rearrange("p a b -> p (a b)")
                )

                # kill the scan state at block boundaries (bid columns)
                nc.scalar.mul(
                    xt[:, SEQ + 1 : SCAN_W : W], xt[:, SEQ + 1 : SCAN_W : W], 0.0
                )

                buf = cp_bufs[i % n_cp]
                if scan:
                    nc.vector.tensor_tensor_scan(
                        out=buf[:, 1:NE],
                        data0=xt[:, 0:SCAN_W],
                        data1=RST[:, :],
                        initial=1.0,
                        op0=mybir.AluOpType.mult,
                        op1=mybir.AluOpType.add,
                    )

                if gather:
                    G = gat_pool.tile([128, NIDX], F32, tag="g")
                    nc.gpsimd.ap_gather(
                        G[:, :], buf[:, :], idx16[:, :],
                        channels=128, num_elems=NE, d=1, num_idxs=NIDX,
                    )
                    g_in = G[:, :]
                else:
                    g_in = WM[:, :]

                if extract_tt:
                    GM = gat_pool.tile([128, NIDX], F32, tag="gm")
                    nc.vector.tensor_tensor(
                        GM[:, :], g_in, WM[:, :], mybir.AluOpType.mult
                    )
                    red_in = GM[:, :]
                else:
                    red_in = g_in
                nc.vector.tensor_reduce(
                    out=ACC[:, i * R : (i + 1) * R, :],
                    in_=red_in.rearrange("p (s l) -> p s l", l=16),
                    op=mybir.AluOpType.add,
                    axis=mybir.AxisListType.X,
                )

            nc.vector.tensor_tensor(
                RL[:, :], ACC[:, :, 0], ACC[:, :, 1], mybir.AluOpType.subtract
            )
            nc.sync.dma_start(out=out_s[:, :], in_=ACC[:, :, 2])
            nc.sync.dma_start(out=out_r[:, :], in_=RL[:, :])

    _split_multi_waits(nc)
    # Raw Bass skips Bacc's codegen pass that fills in .instr bytes for
    # extended-ISA instructions (ap_gather, library load); without it the
    # NEFF compiler fails with "ISA wrong length".
    from concourse.library_overlay import lower_extended_insts

    lower_extended_insts(nc)
    return nc


def make_consts_log():
    iota = np.tile(np.arange(SEQ, dtype=np.float32), (128, 1))
    return {"iotaf": iota}


def _build_nc_log(repeat=1, pool=1, in_bufs=6, l_bufs=4, tiles=None):
    """Log-space architecture: no cumprod scan, no gathers.

    Per 128-row tile: ACT computes L = ln(rates); three one-pass STT
    masked accumulations produce s_bid = sum(L[j<bid]), s_mp =
    sum(L[j<mp]) and r_mp = rates[mp]; tail computes exp() on ACT and
    rate_last = exp(s_mp)*(1-r_mp). `pool` (int or digit-string cycle)
    sets how many of the 3 STTs run on GPSIMD instead of DVE per tile.
    The r_mp STT only needs the raw tile so it runs in parallel with the
    Ln; the two masked-L sums depend on the Ln output.
    """
    import concourse.bass as bass
    import concourse.tile as tile
    from concourse import mybir

    F32 = mybir.dt.float32
    TT = tiles or T_LOG
    pattern = [int(c) for c in str(pool)]

    nc = bass.Bass("TRN2")
    x = nc.dram_tensor("inputs", [TT * 128, W], F32, kind="ExternalInput")
    c_iota = nc.dram_tensor("iotaf", [128, SEQ], F32, kind="ExternalInput")
    out_s = nc.dram_tensor("surv", [128, TT], F32, kind="ExternalOutput")
    out_r = nc.dram_tensor("ratelast", [128, TT], F32, kind="ExternalOutput")

    x_t = x.rearrange("(t p) c -> t p c", p=128)

    with tile.TileContext(nc) as tc:
        with (
            tc.tile_pool(name="inp", bufs=in_bufs) as inp_pool,
            tc.tile_pool(name="lp", bufs=l_bufs) as l_pool,
            tc.tile_pool(name="persist", bufs=1) as persist,
        ):
            IOTA = persist.tile([128, SEQ], F32, tag="iota")
            nc.sync.dma_start(out=IOTA[:, :], in_=c_iota[:, :])

            SB = persist.tile([128, TT], F32, tag="sb")
            SM = persist.tile([128, TT], F32, tag="sm")
            RM = persist.tile([128, TT], F32, tag="rm")

            for i in [i for _ in range(repeat) for i in range(TT)]:
                xt = inp_pool.tile([128, W], F32, tag="xt")
                nc.sync.dma_start(out=xt[:, :], in_=x_t[i, :, :])
                rates = xt[:, 0:SEQ]
                mp = xt[:, SEQ : SEQ + 1]
                bid = xt[:, SEQ + 1 : SEQ + 2]

                L = l_pool.tile([128, SEQ], F32, tag="L")
                nc.scalar.activation(
                    L[:, :], rates, mybir.ActivationFunctionType.Ln
                )

                n_pool = pattern[i % len(pattern)]
                tr1 = l_pool.tile([128, SEQ], F32, tag="tr1")
                tr2 = l_pool.tile([128, SEQ], F32, tag="tr2")
                tr3 = l_pool.tile([128, SEQ], F32, tag="tr3")
                # (engine, in0, scalar, in1, op0, acc, out-trash)
                jobs = [
                    # r_mp: raw-tile dependent only -> Pool first choice
                    (IOTA[:, :], mp, rates, mybir.AluOpType.is_equal, RM, tr1),
                    # s_bid
                    (IOTA[:, :], bid, L[:, :], mybir.AluOpType.is_lt, SB, tr2),
                    # s_mp
                    (IOTA[:, :], mp, L[:, :], mybir.AluOpType.is_lt, SM, tr3),
                ]
                for j, (in0, sc, in1, op0, acc, tr) in enumerate(jobs):
                    eng = nc.gpsimd if j < n_pool else nc.vector
                    eng.scalar_tensor_tensor(
                        out=tr[:, :],
                        in0=in0,
                        scalar=sc,
                        in1=in1,
                        op0=op0,
                        op1=mybir.AluOpType.mult,
                        accum_out=acc[:, i : i + 1],
                    )

            ES = persist.tile([128, TT], F32, tag="es")
            nc.scalar.activation(ES[:, :], SB[:, :], mybir.ActivationFunctionType.Exp)
            EM = persist.tile([128, TT], F32, tag="em")
            nc.scalar.activation(EM[:, :], SM[:, :], mybir.ActivationFunctionType.Exp)
            OM = persist.tile([128, TT], F32, tag="om")
            nc.vector.tensor_scalar(
                out=OM[:, :], in0=RM[:, :], scalar1=-1.0, scalar2=1.0,
                op0=mybir.AluOpType.mult, op1=mybir.AluOpType.add,
            )
            RL = persist.tile([128, TT], F32, tag="rlt")
            nc.vector.tensor_tensor(RL[:, :], EM[:, :], OM[:, :], mybir.AluOpType.mult)
            nc.sync.dma_start(out=out_s[:, :], in_=ES[:, :])
            nc.sync.dma_start(out=out_r[:, :], in_=RL[:, :])

    _split_multi_waits(nc)
    from concourse.library_overlay import lower_extended_insts

    lower_extended_insts(nc)
    return nc


T_LOG = 489  # tiles per core for the log architecture; 489*128 = 62592


def make_consts_f():
    iota = np.tile(np.arange(SEQ, dtype=np.float32), (128, 1))
    wmc = np.zeros((128, 128), np.float32)
    for p in range(128):
        wmc[p, np.arange(R) * 16 + p % 16] = 1.0
    offc = np.zeros((128, R), np.float32)
    for k in range(R):
        offc[:, k] = k * W
    return {"iotaf": iota, "wmc": wmc, "offc": offc}


def _build_nc_f2(
    repeat=1, in_bufs=6, l_bufs=5, nstt=16, ntrash=4, gather=1, strided_extract=0
):
    """Arch F2: like F but the gather results accumulate into a persistent
    [128, T*128] buffer with NO per-tile consumers; extraction + tail run
    once at the end (strided per-partition copies or mask+reduce)."""
    import concourse.bass as bass
    import concourse.tile as tile
    from concourse import mybir, library_config

    F32 = mybir.dt.float32
    I16 = mybir.dt.int16
    NID = 16 * R

    nc = bass.Bass("TRN2")
    x = nc.dram_tensor("inputs", [ROWS_PER_CORE, W], F32, kind="ExternalInput")
    c_iota = nc.dram_tensor("iotaf", [128, SEQ], F32, kind="ExternalInput")
    c_wmc = nc.dram_tensor("wmc", [128, 128], F32, kind="ExternalInput")
    c_offc = nc.dram_tensor("offc", [128, R], F32, kind="ExternalInput")
    out_s = nc.dram_tensor("surv", [128, OUT_COLS], F32, kind="ExternalOutput")
    out_r = nc.dram_tensor("ratelast", [128, OUT_COLS], F32, kind="ExternalOutput")

    x_t = x.rearrange("(t r p) c -> t p r c", p=128, r=R)

    with tile.TileContext(nc) as tc:
        with (
            tc.tile_pool(name="inp", bufs=in_bufs) as inp_pool,
            tc.tile_pool(name="lp", bufs=l_bufs) as l_pool,
            tc.tile_pool(name="gat", bufs=4) as gat_pool,
            tc.tile_pool(name="persist", bufs=1) as persist,
        ):
            IOTA = persist.tile([128, SEQ], F32, tag="iota")
            WMC = persist.tile([128, 128], F32, tag="wmc")
            OFFC = persist.tile([128, R], F32, tag="offc")
            nc.sync.dma_start(out=IOTA[:, :], in_=c_iota[:, :])
            nc.sync.dma_start(out=WMC[:, :], in_=c_wmc[:, :])
            nc.sync.dma_start(out=OFFC[:, :], in_=c_offc[:, :])

            nc.gpsimd.load_library(library_config.ap_gather)

            SB = persist.tile([128, T * R], F32, tag="sb")
            SM = persist.tile([128, T * R], F32, tag="sm")
            GBIG = persist.tile([128, T * NID], F32, tag="gbig")
            if nstt < 16:
                nc.vector.memset(SB[:, :], 0.0)
                nc.vector.memset(SM[:, :], 0.0)
            if not gather:
                nc.vector.memset(GBIG[:, :], 0.0)

            for i in [i for _ in range(repeat) for i in range(T)]:
                xt = inp_pool.tile([128, WIDE], F32, tag="xt")
                xt3 = xt[:, :].rearrange("p (r c) -> p r c", r=R)
                nc.sync.dma_start(out=xt3, in_=x_t[i, :, :, :])

                L = l_pool.tile([128, R, SEQ], F32, tag="L")
                nc.scalar.activation(
                    L[:, :, :], xt3[:, :, 0:SEQ], mybir.ActivationFunctionType.Ln
                )

                idxf = gat_pool.tile([128, R], F32, tag="idxf")
                nc.vector.scalar_tensor_tensor(
                    out=idxf[:, :], in0=xt[:, SEQ : WIDE : W], scalar=0.0,
                    in1=OFFC[:, :], op0=mybir.AluOpType.add,
                    op1=mybir.AluOpType.add,
                )
                idx16 = gat_pool.tile([128, R], I16, tag="idx16")
                nc.scalar.copy(idx16[:, :], idxf[:, :])

                if gather:
                    nc.gpsimd.ap_gather(
                        GBIG[:, i * NID : (i + 1) * NID], xt[:, :], idx16[:, :],
                        channels=128, num_elems=WIDE, d=1, num_idxs=NID,
                    )

                trash = []
                for j in range(ntrash):
                    trj = l_pool.tile([128, SEQ], F32, tag=f"tr{j}")
                    trash.append(trj)
                stt_jobs = []
                for k in range(R):
                    stt_jobs.append((k, SEQ + 1, SB))
                    stt_jobs.append((k, SEQ, SM))
                for jn, (k, sc_col, acc) in enumerate(stt_jobs[:nstt]):
                    nc.vector.scalar_tensor_tensor(
                        out=trash[jn % ntrash][:, :],
                        in0=IOTA[:, :],
                        scalar=xt[:, k * W + sc_col : k * W + sc_col + 1],
                        in1=L[:, k, :],
                        op0=mybir.AluOpType.is_lt,
                        op1=mybir.AluOpType.mult,
                        accum_out=acc[:, i * R + k : i * R + k + 1],
                    )

            # ---- end-of-pass extraction + tail ----
            RM = persist.tile([128, OUT_COLS], F32, tag="rm")
            GB3 = GBIG[:, :].rearrange("p (c l) -> p c l", l=16)
            if strided_extract:
                for l in range(16):
                    nc.vector.tensor_copy(
                        RM[l : 128 : 16, :], GB3[l : 128 : 16, :, l]
                    )
            else:
                GM = persist.tile([128, T * NID], F32, tag="gm")
                wmb = (
                    WMC[:, :]
                    .rearrange("p (one c) -> p one c", one=1)
                    .broadcast_to((128, T, NID))
                )
                nc.vector.tensor_tensor(
                    GM[:, :].rearrange("p (t c) -> p t c", c=NID),
                    GBIG[:, :].rearrange("p (t c) -> p t c", c=NID),
                    wmb,
                    mybir.AluOpType.mult,
                )
                nc.vector.tensor_reduce(
                    out=RM[:, :],
                    in_=GM[:, :].rearrange("p (c l) -> p c l", l=16),
                    op=mybir.AluOpType.add,
                    axis=mybir.AxisListType.X,
                )

            OUTS = persist.tile([128, OUT_COLS], F32, tag="outs")
            OUTR = persist.tile([128, OUT_COLS], F32, tag="outr")
            EM = persist.tile([128, OUT_COLS], F32, tag="em")
            OM = persist.tile([128, OUT_COLS], F32, tag="om")
            nc.scalar.activation(
                OUTS[:, :], SB[:, :], mybir.ActivationFunctionType.Exp
            )
            nc.scalar.activation(EM[:, :], SM[:, :], mybir.ActivationFunctionType.Exp)
            nc.scalar.activation(
                OM[:, :], RM[:, :], mybir.ActivationFunctionType.Copy,
                bias=1.0, scale=-1.0,
            )
            nc.vector.tensor_tensor(
                OUTR[:, :], EM[:, :], OM[:, :], mybir.AluOpType.mult
            )
            nc.sync.dma_start(out=out_s[:, :], in_=OUTS[:, :])
            nc.sync.dma_start(out=out_r[:, :], in_=OUTR[:, :])

    _split_multi_waits(nc)
    from concourse.library_overlay import lower_extended_insts

    lower_extended_insts(nc)
    return nc


def _build_nc_f(repeat=1, in_bufs=5, l_bufs=4, g_bufs=4, nstt=16, ntrash=4, gather=1):
    """Arch F: log-space masked sums on DVE + raw-tile r_mp gather on Pool.

    Per tile (R=8 row-blocks, 1024 rows):
      ACT:  L[k] = ln(rates block k)  (one wide strided op) + idx cast
      DVE:  16 STT masked sums: s_bid[k] = sum(L_k[j<bid_k]),
            s_mp[k] = sum(L_k[j<mp_k]); + extraction of the gather
      Pool: one ap_gather pulls r_mp[k] = xt[302k + mp_k] for all 16
            rows x 8 blocks of each partition group (128 idx)
      tail (per tile, tiny): surv = exp(s_bid); rate_last =
            exp(s_mp)*(1 - r_mp)
    """
    import concourse.bass as bass
    import concourse.tile as tile
    from concourse import mybir, library_config

    F32 = mybir.dt.float32
    I16 = mybir.dt.int16
    NID = 16 * R  # 128 gather slots per partition-group

    nc = bass.Bass("TRN2")
    x = nc.dram_tensor("inputs", [ROWS_PER_CORE, W], F32, kind="ExternalInput")
    c_iota = nc.dram_tensor("iotaf", [128, SEQ], F32, kind="ExternalInput")
    c_wmc = nc.dram_tensor("wmc", [128, 128], F32, kind="ExternalInput")
    c_offc = nc.dram_tensor("offc", [128, R], F32, kind="ExternalInput")
    out_s = nc.dram_tensor("surv", [128, OUT_COLS], F32, kind="ExternalOutput")
    out_r = nc.dram_tensor("ratelast", [128, OUT_COLS], F32, kind="ExternalOutput")

    x_t = x.rearrange("(t r p) c -> t p r c", p=128, r=R)

    with tile.TileContext(nc) as tc:
        with (
            tc.tile_pool(name="inp", bufs=in_bufs) as inp_pool,
            tc.tile_pool(name="lp", bufs=l_bufs) as l_pool,
            tc.tile_pool(name="gat", bufs=g_bufs) as gat_pool,
            tc.tile_pool(name="persist", bufs=1) as persist,
        ):
            IOTA = persist.tile([128, SEQ], F32, tag="iota")
            WMC = persist.tile([128, 128], F32, tag="wmc")
            OFFC = persist.tile([128, R], F32, tag="offc")
            nc.sync.dma_start(out=IOTA[:, :], in_=c_iota[:, :])
            nc.sync.dma_start(out=WMC[:, :], in_=c_wmc[:, :])
            nc.sync.dma_start(out=OFFC[:, :], in_=c_offc[:, :])

            nc.gpsimd.load_library(library_config.ap_gather)

            SB = persist.tile([128, T * R], F32, tag="sb")
            SM = persist.tile([128, T * R], F32, tag="sm")
            OUTS = persist.tile([128, OUT_COLS], F32, tag="outs")
            OUTR = persist.tile([128, OUT_COLS], F32, tag="outr")
            if nstt < 16:
                nc.vector.memset(SB[:, :], 0.0)
                nc.vector.memset(SM[:, :], 0.0)

            for i in [i for _ in range(repeat) for i in range(T)]:
                xt = inp_pool.tile([128, WIDE], F32, tag="xt")
                xt3 = xt[:, :].rearrange("p (r c) -> p r c", r=R)
                nc.sync.dma_start(out=xt3, in_=x_t[i, :, :, :])

                # L = ln(rates), all 8 blocks in one strided ACT op
                L = l_pool.tile([128, R, SEQ], F32, tag="L")
                nc.scalar.activation(
                    L[:, :, :], xt3[:, :, 0:SEQ], mybir.ActivationFunctionType.Ln
                )

                # r_mp gather indices: 302k + mp_k, cast on ACT
                idxf = gat_pool.tile([128, R], F32, tag="idxf")
                nc.vector.scalar_tensor_tensor(
                    out=idxf[:, :], in0=xt[:, SEQ : WIDE : W], scalar=0.0,
                    in1=OFFC[:, :], op0=mybir.AluOpType.add,
                    op1=mybir.AluOpType.add,
                )
                idx16 = gat_pool.tile([128, R], I16, tag="idx16")
                nc.scalar.copy(idx16[:, :], idxf[:, :])

                G = gat_pool.tile([128, NID], F32, tag="g")
                if gather:
                    nc.gpsimd.ap_gather(
                        G[:, :], xt[:, :], idx16[:, :],
                        channels=128, num_elems=WIDE, d=1, num_idxs=NID,
                    )
                else:
                    nc.vector.memset(G[:, :], 0.0)

                # 16 masked-sum STTs (the DVE workhorse)
                trash = []
                for j in range(ntrash):
                    trj = l_pool.tile([128, SEQ], F32, tag=f"tr{j}")
                    trash.append(trj)
                stt_jobs = []
                for k in range(R):
                    stt_jobs.append((k, SEQ + 1, SB))
                for k in range(R):
                    stt_jobs.append((k, SEQ, SM))
                for jn, (k, sc_col, acc) in enumerate(stt_jobs[:nstt]):
                    nc.vector.scalar_tensor_tensor(
                        out=trash[jn % ntrash][:, :],
                        in0=IOTA[:, :],
                        scalar=xt[:, k * W + sc_col : k * W + sc_col + 1],
                        in1=L[:, k, :],
                        op0=mybir.AluOpType.is_lt,
                        op1=mybir.AluOpType.mult,
                        accum_out=acc[:, i * R + k : i * R + k + 1],
                    )
                for jn in range(len(stt_jobs[:nstt]), 16):
                    pass

                # extract r_mp from the wrapped gather
                GM = gat_pool.tile([128, NID], F32, tag="gm")
                nc.vector.tensor_tensor(
                    GM[:, :], G[:, :], WMC[:, :], mybir.AluOpType.mult
                )
                RMt = gat_pool.tile([128, R], F32, tag="rmt")
                nc.vector.tensor_reduce(
                    out=RMt[:, :],
                    in_=GM[:, :].rearrange("p (s l) -> p s l", l=16),
                    op=mybir.AluOpType.add,
                    axis=mybir.AxisListType.X,
                )

                # tail: surv = exp(s_bid); rate_last = exp(s_mp) * (1 - r_mp)
                sl = slice(i * R, (i + 1) * R)
                nc.scalar.activation(
                    OUTS[:, sl], SB[:, sl], mybir.ActivationFunctionType.Exp
                )
                EM = gat_pool.tile([128, R], F32, tag="em")
                nc.scalar.activation(
                    EM[:, :], SM[:, sl], mybir.ActivationFunctionType.Exp
                )
                OM = gat_pool.tile([128, R], F32, tag="om")
                nc.scalar.activation(
                    OM[:, :], RMt[:, :], mybir.ActivationFunctionType.Copy,
                    bias=1.0, scale=-1.0,
                )
                nc.vector.tensor_tensor(
                    OUTR[:, sl], EM[:, :], OM[:, :], mybir.AluOpType.mult
                )

            nc.sync.dma_start(out=out_s[:, :], in_=OUTS[:, :])
            nc.sync.dma_start(out=out_r[:, :], in_=OUTR[:, :])

    _split_multi_waits(nc)
    from concourse.library_overlay import lower_extended_insts

    lower_extended_insts(nc)
    return nc


def _build_nc_micro(repeat=1, op=0, width=300, n=200, nbuf=4, d=1):
    """Microbenchmark: `repeat*n` back-to-back instructions of one kind.

    op: 0=STT(is_lt,mult,accum) 1=scan 2=TT(mult) 3=reduce(seg16)
        4=ACT Ln  5=ap_gather(num_idxs=width)  6=tensor_copy
    """
    import concourse.bass as bass
    import concourse.tile as tile
    from concourse import mybir, library_config

    F32 = mybir.dt.float32
    I16 = mybir.dt.int16

    nc = bass.Bass("TRN2")
    x = nc.dram_tensor("inputs", [128, W], F32, kind="ExternalInput")
    out_s = nc.dram_tensor("surv", [128, 1], F32, kind="ExternalOutput")
    out_r = nc.dram_tensor("ratelast", [128, 1], F32, kind="ExternalOutput")

    with tile.TileContext(nc) as tc:
        with tc.tile_pool(name="p", bufs=1) as pool:
            A = pool.tile([128, max(width, 2415)], F32, tag="a")
            B = pool.tile([128, max(width, 2415)], F32, tag="b")
            S = pool.tile([128, 1], F32, tag="s")
            nc.sync.dma_start(out=A[:, 0:W], in_=x[:, :])
            nc.vector.memset(A[:, W:], 0.5)
            nc.vector.memset(B[:, :], 0.5)
            nc.vector.memset(S[:, :], 3.0)
            IDX = pool.tile([128, max(width // 16, 4)], I16, tag="idx")
            nc.vector.memset(IDX[:, :], 7)
            if op == 5:
                nc.gpsimd.load_library(library_config.ap_gather)
            trash = []
            acc = []
            for j in range(nbuf):
                tr = pool.tile([128, width * d], F32, tag=f"tr{j}")
                trash.append(tr)
                ac = pool.tile([128, 1], F32, tag=f"ac{j}")
                nc.vector.memset(ac[:, :], 0.0)
                nc.vector.memset(tr[:, :], 0.0)
                acc.append(ac)
            for i in range(repeat * n):
                tr = trash[i % nbuf]
                ac = acc[i % nbuf]
                if op == 0:
                    nc.vector.scalar_tensor_tensor(
                        out=tr[:, :], in0=A[:, 0:width], scalar=S[:, :],
                        in1=B[:, 0:width], op0=mybir.AluOpType.is_lt,
                        op1=mybir.AluOpType.mult, accum_out=ac[:, :],
                    )
                elif op == 1:
                    nc.vector.tensor_tensor_scan(
                        out=tr[:, :], data0=A[:, 0:width], data1=B[:, 0:width],
                        initial=1.0, op0=mybir.AluOpType.mult,
                        op1=mybir.AluOpType.add,
                    )
                elif op == 2:
                    nc.vector.tensor_tensor(
                        tr[:, :], A[:, 0:width], B[:, 0:width], mybir.AluOpType.mult
                    )
                elif op == 3:
                    nc.vector.tensor_reduce(
                        out=tr[:, 0 : width // 16],
                        in_=A[:, 0:width].rearrange("p (s l) -> p s l", l=16),
                        op=mybir.AluOpType.add, axis=mybir.AxisListType.X,
                    )
                elif op == 4:
                    nc.scalar.activation(
                        tr[:, :], A[:, 0:width], mybir.ActivationFunctionType.Ln
                    )
                elif op == 5:
                    nc.gpsimd.ap_gather(
                        tr[:, :],
                        B[:, 0 : (2415 // d) * d].rearrange(
                            "p (e dd) -> p e dd", dd=d
                        ),
                        IDX[:, 0 : width // 16],
                        channels=128, num_elems=2415 // d, d=d, num_idxs=width,
                    )
                elif op == 6:
                    nc.vector.tensor_copy(tr[:, :], A[:, 0:width])
            nc.sync.dma_start(out=out_s[:, :], in_=acc[0][:, :])
            nc.sync.dma_start(out=out_r[:, :], in_=trash[0][:, 0:1])

    _split_multi_waits(nc)
    from concourse.library_overlay import lower_extended_insts

    lower_extended_insts(nc)
    return nc


# --- public kernel() configuration ---
# "log":  log-space masked-sum architecture (_build_nc_log)
# "scan": R=8 scan + ap_gather architecture (_build_nc)
KCONFIG = "log"


def _active_rows_per_core():
    return T_LOG * 128 if KCONFIG == "log" else ROWS_PER_CORE


def _build_active(repeat=1):
    if KCONFIG == "log":
        return _build_nc_log(pool=0, repeat=repeat)
    if KCONFIG == "f":
        return _build_nc_f(repeat=repeat)
    if KCONFIG == "f2":
        return _build_nc_f2(repeat=repeat)
    return _build_nc(repeat=repeat)


def _get_nc():
    key = f"nc_{KCONFIG}"
    if key not in _CACHE:
        _CACHE[key] = _build_active()
    return _CACHE[key]


def _shard_inputs(inputs, rows_per_core=None):
    rpc = rows_per_core or _active_rows_per_core()
    total = N_CORES * rpc
    padded = np.empty((total, W), dtype=np.float32)
    padded[: inputs.shape[0]] = inputs
    if total > inputs.shape[0]:
        padded[inputs.shape[0] :, :SEQ] = 1.0
        padded[inputs.shape[0] :, SEQ:] = 0.0
    return [padded[c * rpc : (c + 1) * rpc] for c in range(N_CORES)]


def _active_consts():
    if KCONFIG == "log":
        return make_consts_log()
    if KCONFIG in ("f", "f2"):
        return make_consts_f()
    return make_consts()


def kernel(inputs: np.ndarray):
    from concourse.bass_utils import run_bass_kernel_spmd

    inputs = np.ascontiguousarray(inputs, dtype=np.float32)
    assert inputs.shape == (B, W), inputs.shape

    nc = _get_nc()
    shards = _shard_inputs(inputs)
    consts = _active_consts()
    res = run_bass_kernel_spmd(
        nc,
        [{"inputs": s, **consts} for s in shards],
        core_ids=list(range(N_CORES)),
    )
    surv = np.concatenate(
        [r["surv"].T.reshape(-1, 1) for r in res.results], axis=0
    )[:B]
    rl = np.concatenate(
        [r["ratelast"].T.reshape(-1, 1) for r in res.results], axis=0
    )[:B]
    return surv, rl
